# revision 18
# baseline (speedup 1.0000x reference)
"""Trainium2 Bass kernel for a Mamba-SSD single-step state update.

Computes, for full inputs (B=128, H=64, P=64, N=128):
    dt  = softplus(dt_raw + dt_bias)            # (B, H)
    dA  = exp(-dt * exp(A_log))                 # (B, H)
    new_state[b,h,p,n] = state * dA[b,h] + dt[b,h]*B_in[b,n]*x[b,h,p]
    y[b,h,p] = sum_n new_state[b,h,p,n]*C_in[b,n] + D[h]*x[b,h,p]
returns (y, new_state).

Sharding: pure data-parallel over the batch dim across 8 NeuronCores
(16 batches per core), params replicated. No communication needed.

Per-core layout: the (H*P = 4096) rows of one batch's state are processed
as 32 tiles of (128 rows x 128 state-cols); tile t covers heads 2t, 2t+1.
 - PE builds the rank-1 update dt*x (x) B into PSUM (K=1 matmul)
 - DVE fuses new_state = state*dA + outer  (scalar_tensor_tensor)
 - DVE fuses y = sum_n new_state*C + D*x   (tensor_tensor_reduce, D*x via
   the per-partition initial value of the reduction)
 - dA / D*x per-tile per-partition columns are produced by small PE
   matmuls/transposes at the start; y is transposed back via PE at the end.
"""

import numpy as np

B_FULL, H, PH, NS = 128, 64, 64, 128
N_CORES = 8
NB = B_FULL // N_CORES  # batches per core
T = (H * PH) // 128  # 32 tiles per batch (tile = 2 heads)

_nc_cache = {}


def _split_multi_waits(nc):
    """The walrus build in this env accepts at most ONE sync wait per
    instruction ("Too many sync wait commands"), but Tile attaches one wait
    per dependency semaphore lane. Splice single-wait NoOps in front of any
    instruction carrying more than one wait."""
    from concourse import mybir

    f = nc.m.functions[0]
    for b in f.blocks:
        out = []
        changed = False
        for inst in b.instructions:
            si = inst.sync_info
            if si is not None and len(si.on_wait) > 1:
                waits = list(si.on_wait)
                for w in waits[:-1]:
                    out.append(
                        mybir.InstNoOp(
                            name=f"I-{nc.next_id()}",
                            text_hint="wait_split",
                            engine=inst.engine,
                            sync_info=mybir.SyncInfo(on_wait=[w], on_update=[]),
                            bass_nofuse=True,
                        )
                    )
                inst.sync_info = mybir.SyncInfo(
                    on_wait=[waits[-1]], on_update=list(si.on_update)
                )
                changed = True
            out.append(inst)
        if changed:
            b.instructions = out


def build_nc(nb=NB, split_waits=True):
    import concourse.bass as bass
    import concourse.tile as tile
    from concourse import mybir
    from concourse.masks import make_identity

    f32 = mybir.dt.float32
    AF = mybir.ActivationFunctionType
    OP = mybir.AluOpType

    nc = bass.Bass()

    x_d = nc.dram_tensor("x", [nb, H, PH], f32, kind="ExternalInput")
    b_d = nc.dram_tensor("bin", [nb, NS], f32, kind="ExternalInput")
    c_d = nc.dram_tensor("cin", [nb, NS], f32, kind="ExternalInput")
    dtr_d = nc.dram_tensor("dtraw", [nb, H], f32, kind="ExternalInput")
    st_d = nc.dram_tensor("state", [nb, H, PH, NS], f32, kind="ExternalInput")
    al_d = nc.dram_tensor("alog", [H], f32, kind="ExternalInput")
    dp_d = nc.dram_tensor("dpar", [H], f32, kind="ExternalInput")
    db_d = nc.dram_tensor("dtbias", [H], f32, kind="ExternalInput")
    y_d = nc.dram_tensor("y", [nb, H, PH], f32, kind="ExternalOutput")
    ns_d = nc.dram_tensor("ns", [nb, H, PH, NS], f32, kind="ExternalOutput")

    NBT = nb * T  # total tiles / columns of y_all
    ROWS = nb * 8  # rows of the x-rows layout (each row = 512 x-values)

    with tile.TileContext(nc) as tc:
        with (
            tc.tile_pool(name="consts", bufs=1) as consts,
            tc.tile_pool(name="stp", bufs=2) as stp,
            tc.tile_pool(name="nsp", bufs=2) as nsp,
            tc.tile_pool(name="bcp", bufs=3) as bcp,
            tc.tile_pool(name="po", bufs=4) as pop,
            tc.tile_pool(name="scp", bufs=2) as scp,
            tc.tile_pool(name="prep_ps", bufs=2, space="PSUM") as prep_ps,
        ):
            # ---------------- constants / prep ----------------
            ident = consts.tile([128, 128], f32, tag="ident")
            make_identity(nc, ident)

            # x in rows layout: row r = 8*b + j holds x[b, 8j:8j+8, :] flat.
            x_rows = consts.tile([ROWS, 512], f32, tag="x_rows")
            nc.sync.dma_start(
                out=x_rows,
                in_=x_d[:].rearrange("b h p -> (b h p)").rearrange(
                    "(r c) -> r c", c=512
                ),
            )

            dtr_sb = consts.tile([nb, H], f32, tag="dtr_sb")
            nc.sync.dma_start(out=dtr_sb, in_=dtr_d[:])
            bias_bc = consts.tile([nb, H], f32, tag="bias_bc")
            nc.sync.dma_start(
                out=bias_bc,
                in_=bass.AP(tensor=db_d[:].tensor, offset=0, ap=[[0, nb], [1, H]]),
            )
            alog_bc = consts.tile([nb, H], f32, tag="alog_bc")
            nc.sync.dma_start(
                out=alog_bc,
                in_=bass.AP(tensor=al_d[:].tensor, offset=0, ap=[[0, nb], [1, H]]),
            )

            # dt = softplus(dt_raw + dt_bias) = ln(exp(z) + 1). The argument is
            # bounded (z <~ 6 for this problem) so exp cannot overflow.
            dt_sb = consts.tile([nb, H], f32, tag="dt_sb")
            nc.vector.tensor_add(dt_sb, dtr_sb, bias_bc)
            nc.scalar.activation(dt_sb, dt_sb, AF.Exp)
            nc.scalar.activation(dt_sb, dt_sb, AF.Ln, bias=1.0)
            # dA = exp(-dt * exp(A_log))
            da_sb = consts.tile([nb, H], f32, tag="da_sb")
            nc.scalar.activation(da_sb, alog_bc, AF.Exp)
            nc.vector.tensor_mul(da_sb, da_sb, dt_sb)
            nc.scalar.activation(da_sb, da_sb, AF.Exp, scale=-1.0)

            # transpose dA -> (H, nb)
            ps_dat = prep_ps.tile([H, nb], f32, tag="prep")
            nc.tensor.transpose(ps_dat, da_sb, ident[0:nb, 0:nb])
            dat_sb = consts.tile([H, nb], f32, tag="dat_sb")
            nc.scalar.copy(dat_sb, ps_dat)

            # G_all[h, t, p] = 1 iff h == 2t + p//64 — expands per-head values
            # into per-tile (128,) partition columns via K=64 matmuls at base 0.
            g_all = consts.tile([H, T, 128], f32, tag="g_all")
            nc.gpsimd.memset(g_all, 0.0)
            nc.gpsimd.affine_select(
                out=g_all,
                in_=g_all,
                compare_op=OP.not_equal,
                fill=1.0,
                base=0,
                # expr = h - 2t - blk ; where !=0 keep 0, else fill 1
                pattern=[[-2, T], [-1, 2], [0, 64]],
                channel_multiplier=1,
            )

            # dacol[p, t*nb+b] = dA[b, 2t + (p>=64)]
            ps_dacol = prep_ps.tile([128, nb * T], f32, tag="prep")
            for t in range(T):
                nc.tensor.matmul(
                    ps_dacol[:, t * nb : (t + 1) * nb],
                    g_all[:, t, :],
                    dat_sb,
                    start=True,
                    stop=True,
                )
            dacol = consts.tile([128, nb * T], f32, tag="dacol")
            nc.scalar.copy(dacol, ps_dacol)

            # dt in rows layout (128, 8): [8b+j, h8] = dt[b, 8j+h8]
            dtrow = consts.tile([ROWS, 8], f32, tag="dtrow")
            nc.sync.dma_start(
                out=dtrow, in_=dt_sb[:].rearrange("b (j e) -> b j e", j=8)
            )
            # xd_rows = dt * x in rows layout
            xd_rows = consts.tile([ROWS, 512], f32, tag="xd_rows")
            for h8 in range(8):
                nc.vector.tensor_scalar_mul(
                    xd_rows[:, h8 * 64 : (h8 + 1) * 64],
                    x_rows[:, h8 * 64 : (h8 + 1) * 64],
                    dtrow[:, h8 : h8 + 1],
                )
            # D in rows layout (broadcast over b): [8b+j, h8] = D[8j+h8]
            drow = consts.tile([ROWS, 8], f32, tag="drow")
            nc.sync.dma_start(
                out=drow,
                in_=bass.AP(
                    tensor=dp_d[:].tensor, offset=0, ap=[[0, nb], [8, 8], [1, 8]]
                ),
            )
            dx_rows = consts.tile([ROWS, 512], f32, tag="dx_rows")
            for h8 in range(8):
                nc.vector.tensor_scalar_mul(
                    dx_rows[:, h8 * 64 : (h8 + 1) * 64],
                    x_rows[:, h8 * 64 : (h8 + 1) * 64],
                    drow[:, h8 : h8 + 1],
                )
            # bc[b] = sum_n B[b,n]*C[b,n]; y decomposes as
            # y = dA*(state.C) + bc*dt*x + D*x, avoiding a per-tile reduce.
            bsm = consts.tile([nb, NS], f32, tag="bsm")
            nc.sync.dma_start(out=bsm, in_=b_d[:])
            csm = consts.tile([nb, NS], f32, tag="csm")
            nc.sync.dma_start(out=csm, in_=c_d[:])
            bcp_t = consts.tile([nb, NS], f32, tag="bcprod")
            nc.vector.tensor_mul(bcp_t, bsm, csm)
            bc16 = consts.tile([nb, 1], f32, tag="bc16")
            nc.vector.reduce_sum(bc16, bcp_t, axis=mybir.AxisListType.X)
            bc_rows = consts.tile([ROWS, 1], f32, tag="bc_rows")
            nc.sync.dma_start(
                out=bc_rows,
                in_=bass.AP(
                    tensor=bc16[:].tensor,
                    offset=bc16[:].offset,
                    ap=[[1, nb], [0, 8], [1, 1]],
                ),
            )
            # dxbc_rows = bc*dt*x + D*x (rows layout)
            dxbc_rows = consts.tile([ROWS, 512], f32, tag="dxbc_rows")
            nc.vector.scalar_tensor_tensor(
                out=dxbc_rows,
                in0=xd_rows,
                scalar=bc_rows,
                in1=dx_rows,
                op0=OP.mult,
                op1=OP.add,
            )
            # transpose the 128-col blocks of dxbc_rows / xd_rows; per-tile
            # column (b, t) lives at [t%4][:, 8b + t//4]
            dxbcT = []
            xdcol = []
            for k in range(4):
                ps_t = prep_ps.tile([128, ROWS], f32, tag="prep")
                nc.tensor.transpose(
                    ps_t, dxbc_rows[:, k * 128 : (k + 1) * 128], ident[0:ROWS, 0:ROWS]
                )
                dxk = consts.tile([128, ROWS], f32, tag=f"dxbc{k}")
                nc.scalar.copy(dxk, ps_t)
                dxbcT.append(dxk)
                ps_t2 = prep_ps.tile([128, ROWS], f32, tag="prep")
                nc.tensor.transpose(
                    ps_t2, xd_rows[:, k * 128 : (k + 1) * 128], ident[0:ROWS, 0:ROWS]
                )
                xdk = consts.tile([128, ROWS], f32, tag=f"xdcol{k}")
                nc.scalar.copy(xdk, ps_t2)
                xdcol.append(xdk)

            y_all = consts.tile([128, NBT], f32, tag="y_all")
            scr = consts.tile([128, 128], f32, tag="scr")

            # ---------------- main loop ----------------
            st_view = st_d[:].rearrange("b h p n -> b (h p) n").rearrange(
                "b (t pp) n -> b pp t n", pp=128
            )
            ns_view = ns_d[:].rearrange("b h p n -> b (h p) n").rearrange(
                "b (t pp) n -> b pp t n", pp=128
            )

            for b in range(nb):
                st = stp.tile([128, T, NS], f32, tag="st")
                nc.sync.dma_start(out=st, in_=st_view[b])
                bb = bcp.tile([128, NS], f32, tag="bb")
                nc.sync.dma_start(
                    out=bb,
                    in_=bass.AP(
                        tensor=b_d[:].tensor, offset=b * NS, ap=[[0, 128], [1, NS]]
                    ),
                )
                cb = bcp.tile([128, NS], f32, tag="cb")
                nc.sync.dma_start(
                    out=cb,
                    in_=bass.AP(
                        tensor=c_d[:].tensor, offset=b * NS, ap=[[0, 128], [1, NS]]
                    ),
                )
                ns = nsp.tile([128, T, NS], f32, tag="ns")
                sc_blk = scp.tile([128, T], f32, tag="sc")
                for t in range(T):
                    r = 8 * b + t // 4
                    # outer = (dt*x)_col ⊗ B, built on the scalar engine from
                    # the row-broadcast B tile with a per-partition scale
                    po = pop.tile([128, NS], f32, tag="po")
                    nc.scalar.activation(
                        po,
                        bb,
                        AF.Copy,
                        scale=xdcol[t % 4][:, r : r + 1],
                    )
                    nc.vector.scalar_tensor_tensor(
                        out=ns[:, t, :],
                        in0=st[:, t, :],
                        scalar=dacol[:, t * nb + b : t * nb + b + 1],
                        in1=po,
                        op0=OP.mult,
                        op1=OP.add,
                    )
                    # sc_blk[:, t] = sum_n state*C (row sums via accum_out)
                    nc.vector.scalar_tensor_tensor(
                        out=scr,
                        in0=st[:, t, :],
                        scalar=1.0,
                        in1=cb,
                        op0=OP.mult,
                        op1=OP.mult,
                        accum_out=sc_blk[:, t : t + 1],
                    )
                nc.scalar.dma_start(out=ns_view[b], in_=ns)
                # y columns for this b: dA*sc + (bc*dt*x + D*x)
                tmpb = scp.tile([128, T], f32, tag="tmpb")
                nc.vector.tensor_mul(tmpb, sc_blk, dacol[:, b :: nb])
                for k in range(4):
                    nc.vector.tensor_add(
                        y_all[:, T * b + k : T * (b + 1) : 4],
                        tmpb[:, k :: 4],
                        dxbcT[k][:, 8 * b : 8 * b + T // 4],
                    )

            # ---------------- y assembly ----------------
            # y_all columns are (b, t) pairs; transpose 128-col blocks so DMA
            # writes contiguous 512B rows to HBM.
            y_flat = y_d[:].rearrange("b h p -> (b h p)")
            k0 = 0
            k = 0
            while k0 < NBT:
                cb_w = min(128, NBT - k0)
                ps_y = prep_ps.tile([cb_w, 128], f32, tag="prep")
                nc.tensor.transpose(ps_y, y_all[:, k0 : k0 + cb_w], ident)
                ylin = consts.tile([cb_w, 128], f32, tag=f"ylin{k}")
                nc.scalar.copy(ylin, ps_y)
                bpb = cb_w // T  # batches per block
                nc.scalar.dma_start(
                    out=bass.AP(
                        tensor=y_flat.tensor,
                        offset=k0 // T * H * PH,
                        ap=[[H * PH, bpb], [128, T], [1, 128]],
                    ),
                    in_=ylin,
                )
                k0 += cb_w
                k += 1

    if split_waits:
        _split_multi_waits(nc)
    nc.finalize()
    return nc


def _get_nc(nb=NB):
    if nb not in _nc_cache:
        _nc_cache[nb] = build_nc(nb)
    return _nc_cache[nb]


def _shard(arr, nb, c):
    return np.ascontiguousarray(arr[c * nb : (c + 1) * nb])


def kernel(x_heads, B_in, C_in, dt_raw, state, A_log, D, dt_bias):
    from concourse.bass_utils import run_bass_kernel_spmd

    x_heads = np.ascontiguousarray(np.asarray(x_heads, dtype=np.float32))
    B_in = np.ascontiguousarray(np.asarray(B_in, dtype=np.float32))
    C_in = np.ascontiguousarray(np.asarray(C_in, dtype=np.float32))
    dt_raw = np.ascontiguousarray(np.asarray(dt_raw, dtype=np.float32))
    state = np.ascontiguousarray(np.asarray(state, dtype=np.float32))
    A_log = np.ascontiguousarray(np.asarray(A_log, dtype=np.float32))
    D = np.ascontiguousarray(np.asarray(D, dtype=np.float32))
    dt_bias = np.ascontiguousarray(np.asarray(dt_bias, dtype=np.float32))

    nc = _get_nc(NB)
    in_maps = []
    for c in range(N_CORES):
        in_maps.append(
            {
                "x": _shard(x_heads, NB, c),
                "bin": _shard(B_in, NB, c),
                "cin": _shard(C_in, NB, c),
                "dtraw": _shard(dt_raw, NB, c),
                "state": _shard(state, NB, c),
                "alog": A_log,
                "dpar": D,
                "dtbias": dt_bias,
            }
        )
    res = run_bass_kernel_spmd(nc, in_maps, core_ids=list(range(N_CORES))).results
    y = np.concatenate([r["y"] for r in res], axis=0)
    ns = np.concatenate([r["ns"] for r in res], axis=0)
    return y, ns
